# revision 9
# baseline (speedup 1.0000x reference)
# Multi-head attention kernel for Trainium2, sharded over 8 NeuronCores.
#
# Sharding: core = (batch b, query-chunk qc). Each core handles QB=512 queries
# of one batch, all 12 heads, recomputing the K/V projections for its batch
# (cheaper than cross-core collectives on this fabric: an AllGather of the
# projected K/V measures ~30us + ~50us launch overhead, far more than the
# 15us of redundant fp8 projection compute it would save).
#
# Numerics (rel err budget 2e-2; this design sims at ~9e-3):
#   - q/k path in fp8e4 with DoubleRow matmuls (2x PE): inputs xq/xk and
#     weights wq/wk are fp8; weights are scaled x64 to dodge the e4m3
#     subnormal range, q is stored x8; the 8x score scale is removed for
#     free via the exp activation's scale=1/8.
#   - v path, PV, and output projection stay bf16: fp8 noise on v/ex/o hits
#     the output undamped (sims at 1.5e-2+), so fp8 there is not safe.
#   - d-split layout: host permutes W columns so projection PSUM rows land
#     directly as [32-partition blocks x 2 planes]; scores then contract
#     K=64 as DoubleRow [32p x 2 planes] at 2x rate with no repacking.
#
# Schedule (the baseline ran projections fully before attention, idling the
# Scalar engine for 60us; exp is ~107us of Scalar work and is the binding
# engine once PE is cut down):
#   - kt-blocked attention: 4 blocks of 4 key-tiles; each block sweeps all 6
#     head pairs, accumulating o (+ ones-column denominator row) in PSUM and
#     flushing per-block into an SBUF f32 accumulator on the otherwise-idle
#     GpSimd engine. This keeps live PSUM at 2 score slots + 4 o slots = 8
#     banks and lets the v projection spread across blocks instead of
#     serializing before attention.
#   - k projection is emitted per d-split tile (g-major) so head pairs 0/1
#     can start scoring after ~1/3 of k proj; remaining k/v projection
#     interleaves into the PE slack of the Scalar-bound attention blocks.
#   - softmax normalization (reciprocal + PE broadcast + DVE multiply) is
#     pipelined per head pair right after its last flush, so the tail is
#     only the output projection.

import numpy as np
from contextlib import ExitStack

import concourse.bass as bass
import concourse.mybir as mybir
import concourse.tile as tile
from concourse import bacc
from concourse.bass_utils import run_bass_kernel_spmd

F32 = mybir.dt.float32
BF16 = mybir.dt.bfloat16
F8 = mybir.dt.float8e4
DR = mybir.MatmulPerfMode.DoubleRow
P = 128
E = 768
S = 2048
B = 2
H = 12
D = 64
QB = 512          # queries per core
NCORES = 8
EC = E // P       # 6 e-chunks
KT = S // P       # 16 key tiles
MT = E // P       # 6 e-chunks of proj output
NG = 4            # d-split head groups (head = 3*g + m, m in 0..2)
PT = 8            # projection tiles of 96 rows (bases 96+ are not encodable)
PR = 96           # rows per projection tile
NC4 = S // 512    # 4 n-slices of k
KB = 4            # key tiles per attention block
NBLK = KT // KB   # 4 blocks
WS = 64.0         # fp8 weight scale (avoids e4m3 subnormals)
QS = 8.0          # qT8 storage scale; removed via exp scale


def build_nc():
    nc = bacc.Bacc("TRN2", debug=False)

    xq = nc.dram_tensor("xq", (E, QB), F8, kind="ExternalInput")    # query[b,chunk].T fp8
    xk = nc.dram_tensor("xk", (E, S), F8, kind="ExternalInput")     # key[b].T fp8
    xv = nc.dram_tensor("xv", (E, S), BF16, kind="ExternalInput")   # value[b].T
    wq = nc.dram_tensor("wq", (E, E), F8, kind="ExternalInput")     # d-split cols, x(1/sqrt(D))x64
    wk = nc.dram_tensor("wk", (E, E), F8, kind="ExternalInput")     # d-split cols, x64
    wv = nc.dram_tensor("wv", (E, E), BF16, kind="ExternalInput")
    wo = nc.dram_tensor("wo", (E, E), BF16, kind="ExternalInput")
    bq = nc.dram_tensor("bq", (P, PT), F32, kind="ExternalInput")   # d-split rows, x8
    bk = nc.dram_tensor("bk", (P, PT), F32, kind="ExternalInput")   # d-split rows
    bo = nc.dram_tensor("bo", (P, E), F32, kind="ExternalInput")    # bv@Wo + bo, broadcast
    seld = nc.dram_tensor("seld", (38, H * D), BF16, kind="ExternalInput")  # head-broadcast selector
    out = nc.dram_tensor("out", (QB, E), F32, kind="ExternalOutput")

    with tile.TileContext(nc) as tc:
        with ExitStack() as ctx:
            _emit(ctx, tc, nc, xq, xk, xv, wq, wk, wv, wo, bq, bk, bo, seld, out)
    nc.compile()
    return nc


def _emit(ctx, tc, nc, xq, xk, xv, wq, wk, wv, wo, bq, bk, bo, seld, out):
    mult = mybir.AluOpType.mult
    add = mybir.AluOpType.add

    persist = ctx.enter_context(tc.tile_pool(name="persist", bufs=1))
    wpool = ctx.enter_context(tc.tile_pool(name="wpool", bufs=1))
    xpool = ctx.enter_context(tc.tile_pool(name="xpool", bufs=1))
    xvpool = ctx.enter_context(tc.tile_pool(name="xvpool", bufs=3))
    expool = ctx.enter_context(tc.tile_pool(name="expool", bufs=4))
    outpool = ctx.enter_context(tc.tile_pool(name="outpool", bufs=2))
    psS = ctx.enter_context(tc.tile_pool(name="psS", bufs=2, space="PSUM"))  # [128,2,512] scores/proj
    psO = ctx.enter_context(tc.tile_pool(name="psO", bufs=4, space="PSUM"))  # [65,512] o accum / bc

    # persistent SBUF
    qT8 = persist.tile([P, 2, NG, QB], F8)         # [32m+dm, plane j, group g, q] = 8*q
    kT8 = persist.tile([P, 2, NG, S], F8)          # [32m+dm, j, g, key] = k
    v_sb = persist.tile([P, KT, H, D + 1], BF16)   # v + ones column per head
    o_acc = persist.tile([D + 1, H, QB], F32)      # flushed o (+denominator in row D)
    o_all = persist.tile([P, H // 2, QB], BF16)    # normalized o, pairs in partition halves
    dens = persist.tile([38, QB], F32)             # denominators: halves at rows 0-5 / 32-37
    drec2 = persist.tile([38, QB], BF16)           # their reciprocals
    sel_sb = persist.tile([38, H * D], BF16)       # head-broadcast selector
    bq_sb = persist.tile([P, PT], F32)
    bk_sb = persist.tile([P, PT], F32)
    bo_sb = persist.tile([P, E], F32)

    wq_t = wpool.tile([P, EC, E], F8, tag="wq")
    wk_t = wpool.tile([P, EC, E], F8, tag="wk")
    wv_t = wpool.tile([P, EC, E], BF16, tag="wv")
    wo_t = wpool.tile([P, EC, E], BF16, tag="wo")
    xq_t = xpool.tile([P, EC, QB], F8, tag="xq")
    xk_t = xpool.tile([P, EC, S], F8, tag="xk")

    # --- input DMAs: three queues in parallel so k-proj inputs land early ---
    for ec in range(EC):
        nc.sync.dma_start(wq_t[:, ec, :], wq[ec * P:(ec + 1) * P, :])
        nc.sync.dma_start(xq_t[:, ec, :], xq[ec * P:(ec + 1) * P, :])
        nc.scalar.dma_start(wk_t[:, ec, :], wk[ec * P:(ec + 1) * P, :])
    for ec in range(3):
        nc.scalar.dma_start(xk_t[:, ec, :], xk[ec * P:(ec + 1) * P, :])
        nc.gpsimd.dma_start(xk_t[:, 3 + ec, :], xk[(3 + ec) * P:(4 + ec) * P, :])
    nc.gpsimd.dma_start(bq_sb[:], bq[:])
    nc.gpsimd.dma_start(bk_sb[:], bk[:])
    for ec in range(EC):
        nc.gpsimd.dma_start(wv_t[:, ec, :], wv[ec * P:(ec + 1) * P, :])
    nc.gpsimd.dma_start(bo_sb[:], bo[:])
    nc.gpsimd.dma_start(sel_sb[:], seld[:])

    nc.vector.memset(v_sb[:, :, :, D], 1.0)

    # --- q projection: fp8 DoubleRow, output straight into d-split layout ---
    for t in range(PT):
        g, j = t // 2, t % 2
        ps = psS.tile([P, 2, 512], F32, tag="sc", name="qproj")
        for e in range(3):
            nc.tensor.matmul(ps[0:PR, 0, :], wq_t[:, 2 * e:2 * e + 2, t * PR:(t + 1) * PR],
                             xq_t[:, 2 * e:2 * e + 2, :],
                             start=(e == 0), stop=(e == 2), perf_mode=DR)
        nc.vector.tensor_scalar(qT8[0:PR, j, g, :], ps[0:PR, 0, :], QS / WS,
                                bq_sb[0:PR, t:t + 1], mult, add)

    def emit_kproj(t):
        g, j = t // 2, t % 2
        for n4 in range(NC4):
            ps = psS.tile([P, 2, 512], F32, tag="sc", name="kproj")
            for e in range(3):
                nc.tensor.matmul(ps[0:PR, 0, :], wk_t[:, 2 * e:2 * e + 2, t * PR:(t + 1) * PR],
                                 xk_t[:, 2 * e:2 * e + 2, n4 * 512:(n4 + 1) * 512],
                                 start=(e == 0), stop=(e == 2), perf_mode=DR)
            nc.vector.tensor_scalar(kT8[0:PR, j, g, n4 * 512:(n4 + 1) * 512], ps[0:PR, 0, :],
                                    1.0 / WS, bk_sb[0:PR, t:t + 1], mult, add)

    def emit_vproj(kt):
        xv_t = xvpool.tile([P, EC, P], BF16, tag="xv")
        nc.gpsimd.dma_start(xv_t[:], xv[:, kt * P:(kt + 1) * P].rearrange("(ec p) s -> p ec s", p=P))
        psv = psS.tile([P, 2, 512], F32, tag="sc", name="vproj")
        fl = psv.rearrange("p a b -> p (a b)")
        for ec in range(EC):
            nc.tensor.matmul(fl[:, 0:512], xv_t[:, ec, :], wv_t[:, ec, 0:512],
                             start=(ec == 0), stop=(ec == EC - 1))
            nc.tensor.matmul(fl[:, 512:768], xv_t[:, ec, :], wv_t[:, ec, 512:768],
                             start=(ec == 0), stop=(ec == EC - 1))
        nc.vector.tensor_copy(v_sb[:, kt, :, 0:D], fl[:, 0:768].rearrange("p (h d) -> p h d", d=D))

    # head pairs 0/1 need only d-split k tiles 0,1 (group g=0)
    emit_kproj(0)
    emit_kproj(1)

    def flush_norm(half):
        # one reciprocal across 6 heads (multi-lane), then per-head PE
        # broadcast via the selector and DVE multiply
        r0 = 32 * half
        with nc.allow_low_precision(reason="1/denom in bf16: feeds a bf16 broadcast anyway"):
            nc.vector.reciprocal(drec2[r0:r0 + 6, :], dens[r0:r0 + 6, :])
        for hh in range(6):
            h = 6 * half + hh
            hp, i = h // 2, h % 2
            bc = psO.tile([D + 1, 512], F32, tag="po", name=f"bc{h}")
            nc.tensor.matmul(bc[0:D, :], sel_sb[r0:r0 + 6, h * D:(h + 1) * D],
                             drec2[r0:r0 + 6, :], start=True, stop=True)
            nc.vector.tensor_tensor(o_all[64 * i:64 * i + D, hp, :], bc[0:D, :],
                                    o_acc[0:D, h, :], mult)

    def emit_norm(hp):
        # stage this pair's denominator rows into the multi-partition dens tile
        r = 32 * (hp // 3) + 2 * (hp % 3)
        nc.sync.dma_start(dens[r:r + 2, :],
                          o_acc[D:D + 1, 2 * hp:2 * hp + 2, :])

    def emit_scores(hp, kt):
        st = psS.tile([P, 2, 512], F32, tag="sc", name="sc")
        for i in range(2):
            h = 2 * hp + i
            g, m = h // 3, h % 3
            nc.tensor.matmul(st[:, i, :],
                             kT8[32 * m:32 * m + 32, :, g, kt * P:(kt + 1) * P],
                             qT8[32 * m:32 * m + 32, :, g, :],
                             start=True, stop=True, perf_mode=DR)
        ex = expool.tile([P, 2, 512], BF16, tag="ex")
        nc.scalar.activation(ex[:, :, :], st[:, :, :],
                             mybir.ActivationFunctionType.Exp, scale=1.0 / QS)
        return ex

    def emit_pv(hp, kt, ex, o_ps, start, stop):
        for i in range(2):
            nc.tensor.matmul(o_ps[i][:, :], v_sb[:, kt, 2 * hp + i, :], ex[:, i, :],
                             start=start, stop=stop)

    # --- kt-blocked attention, all head pairs per block.
    # PV is software-pipelined one key-tile behind scores/exp so the in-order
    # PE stream never stalls on the Scalar engine; remaining k/v projection
    # work is emitted in the stall-free gaps. ---
    BLOCKS = [(0, 4), (4, 10), (10, 16)]
    # extra work emitted after (b, hp)'s flush: k-proj tiles / v-proj tiles
    post = {
        (0, 0): lambda: (emit_kproj(2), emit_kproj(3)),
        (0, 1): lambda: (emit_kproj(4), emit_kproj(5), emit_vproj(4)),
        (0, 2): lambda: (emit_kproj(6), emit_kproj(7), emit_vproj(5)),
        (0, 3): lambda: emit_vproj(6),
        (0, 4): lambda: emit_vproj(7),
        (0, 5): lambda: emit_vproj(8),
        (1, 0): lambda: (nc.gpsimd.dma_start(wo_t[:], wo[:].rearrange("(ec p) m -> p ec m", p=P)),
                         emit_vproj(10)),
        (1, 1): lambda: emit_vproj(11),
        (1, 2): lambda: emit_vproj(12),
        (1, 3): lambda: emit_vproj(13),
        (1, 4): lambda: emit_vproj(14),
        (1, 5): lambda: emit_vproj(15),
    }
    for b, (k0, k1) in enumerate(BLOCKS):
        for hp in range(H // 2):
            o_ps = [psO.tile([D + 1, 512], F32, tag="po", name=f"o{b}_{hp}_{i}")
                    for i in range(2)]
            if b == 0 and hp == 0:
                # startup special case: all scores/exp first (Scalar engine
                # starts as soon as k tiles 0,1 land), v-proj + PV after
                exs = [emit_scores(hp, kt) for kt in range(k0, k1)]
                for kt in range(k0, k1):
                    emit_vproj(kt)
                    emit_pv(hp, kt, exs[kt - k0], o_ps,
                            start=(kt == k0), stop=(kt == k1 - 1))
            else:
                if b == 1 and hp == 0:
                    emit_vproj(9)   # kt9 is consumed at the end of this hp's span
                prev = None
                for kt in range(k0, k1):
                    ex = emit_scores(hp, kt)
                    if prev is not None:
                        emit_pv(hp, prev, prev_ex, o_ps, start=(prev == k0), stop=False)
                    prev, prev_ex = kt, ex
                emit_pv(hp, prev, prev_ex, o_ps, start=(prev == k0), stop=True)
            # flush block-partial o (+den row) into the SBUF f32 accumulator
            for i in range(2):
                h = 2 * hp + i
                if b == 0:
                    nc.vector.tensor_copy(o_acc[:, h, :], o_ps[i][:, :])
                else:
                    nc.vector.tensor_tensor(o_acc[:, h, :], o_ps[i][:, :], o_acc[:, h, :], add)
            fn = post.get((b, hp))
            if fn is not None:
                fn()
            if b == len(BLOCKS) - 1:
                emit_norm(hp)
                if hp == 2 or hp == H // 2 - 1:
                    flush_norm(hp // 3)

    # --- output projection ---
    ST = QB // P
    for st4 in range(ST):
        op = psS.tile([P, 2, 512], F32, tag="sc", name="oproj")
        opf = op.rearrange("p a b -> p (a b)")
        for hp in range(H // 2):
            first = (hp == 0)
            last = (hp == H // 2 - 1)
            nc.tensor.matmul(opf[:, 0:512], o_all[:, hp, st4 * P:(st4 + 1) * P],
                             wo_t[:, hp, 0:512], start=first, stop=last)
            nc.tensor.matmul(opf[:, 512:768], o_all[:, hp, st4 * P:(st4 + 1) * P],
                             wo_t[:, hp, 512:768], start=first, stop=last)
        out_sb = outpool.tile([P, E], F32, tag="outsb")
        nc.vector.tensor_tensor(out_sb[:], opf[:, 0:768], bo_sb[:], add)
        nc.sync.dma_start(out[st4 * P:(st4 + 1) * P, :], out_sb[:])


_NC_CACHE = None


def _get_nc():
    global _NC_CACHE
    if _NC_CACHE is None:
        _NC_CACHE = build_nc()
    return _NC_CACHE


def _dsplit_perm():
    """col i = t*96 + m*32 + dm  <-  head (3*(t//2)+m), d (32*(t%2)+dm)."""
    perm = np.empty(E, dtype=np.int64)
    i = 0
    for t in range(PT):
        g, j = t // 2, t % 2
        for m in range(3):
            for dm in range(32):
                perm[i] = (3 * g + m) * D + 32 * j + dm
                i += 1
    return perm


def make_in_maps(query, key_, value, Wq, bq, Wk, bk, Wv, bv, Wo, bo):
    """Host-side sharding + layout prep. Returns list of 8 input dicts."""
    import ml_dtypes
    BF = ml_dtypes.bfloat16
    F8NP = mybir.dt.np(F8)

    query = np.asarray(query, dtype=np.float32)
    key_ = np.asarray(key_, dtype=np.float32)
    value = np.asarray(value, dtype=np.float32)
    scale = 1.0 / np.sqrt(np.float32(D))
    perm = _dsplit_perm()

    wq_eff = np.ascontiguousarray(np.transpose(np.asarray(Wq, np.float32), (1, 0, 2)).reshape(E, E)) * scale
    wk_eff = np.ascontiguousarray(np.transpose(np.asarray(Wk, np.float32), (1, 0, 2)).reshape(E, E))
    wq_f = np.ascontiguousarray(wq_eff[:, perm] * WS).astype(F8NP)
    wk_f = np.ascontiguousarray(wk_eff[:, perm] * WS).astype(F8NP)
    wv_f = np.ascontiguousarray(np.transpose(np.asarray(Wv, np.float32), (1, 0, 2)).reshape(E, E)).astype(BF)
    wo_f = np.ascontiguousarray(np.asarray(Wo, np.float32)).astype(BF)

    bq_eff = np.asarray(bq, np.float32).reshape(E) * scale * QS
    bk_eff = np.asarray(bk, np.float32).reshape(E)
    bq_f = np.zeros((P, PT), np.float32)
    bk_f = np.zeros((P, PT), np.float32)
    bq_f[0:PR, :] = bq_eff[perm].reshape(PT, PR).T
    bk_f[0:PR, :] = bk_eff[perm].reshape(PT, PR).T
    bv_f = np.asarray(bv, np.float32).reshape(E)
    bo_eff = np.tile((bv_f @ wo_f.astype(np.float32) + np.asarray(bo, np.float32)).reshape(1, E), (P, 1)).copy()

    sel_np = np.zeros((38, H * D), np.float32)
    for h in range(H):
        sel_np[32 * (h // 6) + h % 6, h * D:(h + 1) * D] = 1.0
    sel_np = sel_np.astype(BF)

    xk_t = [np.ascontiguousarray(key_[b].T).astype(F8NP) for b in range(B)]
    xv_t = [np.ascontiguousarray(value[b].T).astype(BF) for b in range(B)]

    in_maps = []
    for core in range(NCORES):
        b = core // (NCORES // B)
        qc = core % (NCORES // B)
        xq_f = np.ascontiguousarray(query[b, qc * QB:(qc + 1) * QB, :].T).astype(F8NP)
        in_maps.append({
            "xq": xq_f, "xk": xk_t[b], "xv": xv_t[b],
            "wq": wq_f, "wk": wk_f, "wv": wv_f, "wo": wo_f,
            "bq": bq_f, "bk": bk_f, "bo": bo_eff, "seld": sel_np,
        })
    return in_maps


def assemble(results):
    outp = np.empty((B, S, E), dtype=np.float32)
    for core in range(NCORES):
        b = core // (NCORES // B)
        qc = core % (NCORES // B)
        outp[b, qc * QB:(qc + 1) * QB, :] = results[core]["out"]
    return outp


def kernel(query, key_, value, Wq, bq, Wk, bk, Wv, bv, Wo, bo):
    nc = _get_nc()
    in_maps = make_in_maps(query, key_, value, Wq, bq, Wk, bk, Wv, bv, Wo, bo)
    res = run_bass_kernel_spmd(nc, in_maps, core_ids=list(range(NCORES)))
    return assemble(res.results)


# revision 13
# speedup vs baseline: 1.1048x; 1.1048x over previous
# Multi-head attention kernel for Trainium2, sharded over 8 NeuronCores.
#
# Sharding: core = (batch b, query-chunk qc). Each core handles QB=512 queries
# of one batch, all 12 heads, recomputing the K/V projections for its batch
# (cheaper than cross-core collectives on this fabric: an AllGather of the
# projected K/V measures ~30us + ~50us launch overhead, far more than the
# 15us of redundant fp8 projection compute it would save).
#
# Numerics (rel err budget 2e-2; this design sims at ~9e-3):
#   - q/k path in fp8e4 with DoubleRow matmuls (2x PE): inputs xq/xk and
#     weights wq/wk are fp8; weights are scaled x64 to dodge the e4m3
#     subnormal range, q is stored x8; the 8x score scale is removed for
#     free via the exp activation's scale=1/8.
#   - v path, PV, and output projection stay bf16: fp8 noise on v/ex/o hits
#     the output undamped (sims at 1.5e-2+), so fp8 there is not safe.
#   - d-split layout: host permutes W columns so projection PSUM rows land
#     directly as [32-partition blocks x 2 planes]; scores then contract
#     K=64 as DoubleRow [32p x 2 planes] at 2x rate with no repacking.
#
# Schedule (the baseline ran projections fully before attention, idling the
# Scalar engine for 60us; exp is ~107us of Scalar work and is the binding
# engine once PE is cut down):
#   - kt-blocked attention: 4 blocks of 4 key-tiles; each block sweeps all 6
#     head pairs, accumulating o (+ ones-column denominator row) in PSUM and
#     flushing per-block into an SBUF f32 accumulator on the otherwise-idle
#     GpSimd engine. This keeps live PSUM at 2 score slots + 4 o slots = 8
#     banks and lets the v projection spread across blocks instead of
#     serializing before attention.
#   - k projection is emitted per d-split tile (g-major) so head pairs 0/1
#     can start scoring after ~1/3 of k proj; remaining k/v projection
#     interleaves into the PE slack of the Scalar-bound attention blocks.
#   - softmax normalization (reciprocal + PE broadcast + DVE multiply) is
#     pipelined per head pair right after its last flush, so the tail is
#     only the output projection.

import numpy as np
from contextlib import ExitStack

import concourse.bass as bass
import concourse.mybir as mybir
import concourse.tile as tile
from concourse import bacc
from concourse.bass_utils import run_bass_kernel_spmd

F32 = mybir.dt.float32
BF16 = mybir.dt.bfloat16
F8 = mybir.dt.float8e4
DR = mybir.MatmulPerfMode.DoubleRow
P = 128
E = 768
S = 2048
B = 2
H = 12
D = 64
QB = 512          # queries per core
NCORES = 8
EC = E // P       # 6 e-chunks
KT = S // P       # 16 key tiles
MT = E // P       # 6 e-chunks of proj output
NG = 4            # d-split head groups (head = 3*g + m, m in 0..2)
PT = 8            # projection tiles of 96 rows (bases 96+ are not encodable)
PR = 96           # rows per projection tile
NC4 = S // 512    # 4 n-slices of k
KB = 4            # key tiles per attention block
NBLK = KT // KB   # 4 blocks
WS = 64.0         # fp8 weight scale (avoids e4m3 subnormals)
QS = 8.0          # qT8 storage scale; removed via exp scale


def build_nc():
    nc = bacc.Bacc("TRN2", debug=False)

    xq = nc.dram_tensor("xq", (E, QB), F8, kind="ExternalInput")    # query[b,chunk].T fp8
    xk = nc.dram_tensor("xk", (E, S), F8, kind="ExternalInput")     # key[b].T fp8
    xv = nc.dram_tensor("xv", (E, S), BF16, kind="ExternalInput")   # value[b].T
    wq = nc.dram_tensor("wq", (E, E), F8, kind="ExternalInput")     # d-split cols, x(1/sqrt(D))x64
    wk = nc.dram_tensor("wk", (E, E), F8, kind="ExternalInput")     # d-split cols, x64
    wv = nc.dram_tensor("wv", (E, E), BF16, kind="ExternalInput")
    wo = nc.dram_tensor("wo", (E, E), BF16, kind="ExternalInput")
    bq = nc.dram_tensor("bq", (P, PT), F32, kind="ExternalInput")   # d-split rows, x8
    bk = nc.dram_tensor("bk", (P, PT), F32, kind="ExternalInput")   # d-split rows
    bo = nc.dram_tensor("bo", (P, E), F32, kind="ExternalInput")    # bv@Wo + bo, broadcast
    seld = nc.dram_tensor("seld", (38, H * D), BF16, kind="ExternalInput")  # head-broadcast selector
    out = nc.dram_tensor("out", (QB, E), F32, kind="ExternalOutput")

    with tile.TileContext(nc) as tc:
        with ExitStack() as ctx:
            _emit(ctx, tc, nc, xq, xk, xv, wq, wk, wv, wo, bq, bk, bo, seld, out)
    nc.compile()
    return nc


def _emit(ctx, tc, nc, xq, xk, xv, wq, wk, wv, wo, bq, bk, bo, seld, out):
    mult = mybir.AluOpType.mult
    add = mybir.AluOpType.add

    persist = ctx.enter_context(tc.tile_pool(name="persist", bufs=1))
    wpool = ctx.enter_context(tc.tile_pool(name="wpool", bufs=1))
    xpool = ctx.enter_context(tc.tile_pool(name="xpool", bufs=1))
    xvpool = ctx.enter_context(tc.tile_pool(name="xvpool", bufs=3))
    expool = ctx.enter_context(tc.tile_pool(name="expool", bufs=16))
    outpool = ctx.enter_context(tc.tile_pool(name="outpool", bufs=2))
    psS = ctx.enter_context(tc.tile_pool(name="psS", bufs=2, space="PSUM"))  # [128,2,512] scores/proj
    psO = ctx.enter_context(tc.tile_pool(name="psO", bufs=4, space="PSUM"))  # [65,512] o accum / bc

    # persistent SBUF
    qT8 = persist.tile([P, 2, NG, QB], F8)         # [32m+dm, plane j, group g, q] = 8*q
    kT8 = persist.tile([P, 2, NG, S], F8)          # [32m+dm, j, g, key] = k
    v_sb = persist.tile([P, KT, H, D + 1], BF16)   # v + ones column per head
    o_acc = persist.tile([D + 1, H, QB], F32)      # flushed o (+denominator in row D)
    o_all = persist.tile([P, H // 2, QB], BF16)    # normalized o, pairs in partition halves
    dens = persist.tile([38, QB], F32)             # denominators: halves at rows 0-5 / 32-37
    drec2 = persist.tile([38, QB], BF16)           # their reciprocals
    sel_sb = persist.tile([38, H * D], BF16)       # head-broadcast selector
    bq_sb = persist.tile([P, PT], F32)
    bk_sb = persist.tile([P, PT], F32)
    bo_sb = persist.tile([P, E], F32)

    wq_t = wpool.tile([P, EC, E], F8, tag="wq")
    wk_t = wpool.tile([P, EC, E], F8, tag="wk")
    wv_t = wpool.tile([P, EC, E], BF16, tag="wv")
    wo_t = wpool.tile([P, EC, E], BF16, tag="wo")
    xq_t = xpool.tile([P, EC, QB], F8, tag="xq")
    xk_t = xpool.tile([P, EC, S], F8, tag="xk")

    # --- input DMAs: three queues in parallel so k-proj inputs land early ---
    for ec in range(EC):
        nc.sync.dma_start(wq_t[:, ec, :], wq[ec * P:(ec + 1) * P, :])
        nc.sync.dma_start(xq_t[:, ec, :], xq[ec * P:(ec + 1) * P, :])
        nc.scalar.dma_start(wk_t[:, ec, :], wk[ec * P:(ec + 1) * P, :])
    for ec in range(3):
        nc.scalar.dma_start(xk_t[:, ec, :], xk[ec * P:(ec + 1) * P, :])
        nc.gpsimd.dma_start(xk_t[:, 3 + ec, :], xk[(3 + ec) * P:(4 + ec) * P, :])
    nc.gpsimd.dma_start(bq_sb[:], bq[:])
    nc.gpsimd.dma_start(bk_sb[:], bk[:])
    for ec in range(EC):
        nc.gpsimd.dma_start(wv_t[:, ec, :], wv[ec * P:(ec + 1) * P, :])
    nc.gpsimd.dma_start(bo_sb[:], bo[:])
    nc.gpsimd.dma_start(sel_sb[:], seld[:])

    nc.vector.memset(v_sb[:, :, :, D], 1.0)

    # --- PE warm-up: ~10 dummy matmuls so the pstate ramp (0.65->2.4GHz
    # after ~3us of continuous work) completes before the real projections ---
    wu = persist.tile([P, 512], BF16)
    nc.vector.memset(wu[:], 0.0)
    wups = psS.tile([P, 2, 512], F32, tag="sc", name="warm")
    for _ in range(10):
        nc.tensor.matmul(wups[:, 0, :], wu[:, 0:128], wu[:], start=True, stop=True)

    def emit_qproj(t):
        g, j = t // 2, t % 2
        ps = psS.tile([P, 2, 512], F32, tag="sc", name="qproj")
        for e in range(3):
            nc.tensor.matmul(ps[0:PR, 0, :], wq_t[:, 2 * e:2 * e + 2, t * PR:(t + 1) * PR],
                             xq_t[:, 2 * e:2 * e + 2, :],
                             start=(e == 0), stop=(e == 2), perf_mode=DR)
        nc.vector.tensor_scalar(qT8[0:PR, j, g, :], ps[0:PR, 0, :], QS / WS,
                                bq_sb[0:PR, t:t + 1], mult, add)

    def emit_kproj(t, n4list=range(NC4)):
        g, j = t // 2, t % 2
        for n4 in n4list:
            ps = psS.tile([P, 2, 512], F32, tag="sc", name="kproj")
            for e in range(3):
                nc.tensor.matmul(ps[0:PR, 0, :], wk_t[:, 2 * e:2 * e + 2, t * PR:(t + 1) * PR],
                                 xk_t[:, 2 * e:2 * e + 2, n4 * 512:(n4 + 1) * 512],
                                 start=(e == 0), stop=(e == 2), perf_mode=DR)
            nc.vector.tensor_scalar(kT8[0:PR, j, g, n4 * 512:(n4 + 1) * 512], ps[0:PR, 0, :],
                                    1.0 / WS, bk_sb[0:PR, t:t + 1], mult, add)

    def emit_vproj(kt):
        xv_t = xvpool.tile([P, EC, P], BF16, tag="xv")
        nc.gpsimd.dma_start(xv_t[:], xv[:, kt * P:(kt + 1) * P].rearrange("(ec p) s -> p ec s", p=P))
        psv = psS.tile([P, 2, 512], F32, tag="sc", name="vproj")
        fl = psv.rearrange("p a b -> p (a b)")
        for ec in range(EC):
            nc.tensor.matmul(fl[:, 0:512], xv_t[:, ec, :], wv_t[:, ec, 0:512],
                             start=(ec == 0), stop=(ec == EC - 1))
            nc.tensor.matmul(fl[:, 512:768], xv_t[:, ec, :], wv_t[:, ec, 512:768],
                             start=(ec == 0), stop=(ec == EC - 1))
        nc.vector.tensor_copy(v_sb[:, kt, :, 0:D], fl[:, 0:768].rearrange("p (h d) -> p h d", d=D))

    def flush_norm(half):
        r0 = 32 * half
        with nc.allow_low_precision(reason="1/denom in bf16: feeds a bf16 broadcast anyway"):
            nc.vector.reciprocal(drec2[r0:r0 + 6, :], dens[r0:r0 + 6, :])
        for hh in range(6):
            h = 6 * half + hh
            hp, i = h // 2, h % 2
            bc = psO.tile([D + 1, 512], F32, tag="po", name=f"bc{h}")
            nc.tensor.matmul(bc[0:D, :], sel_sb[r0:r0 + 6, h * D:(h + 1) * D],
                             drec2[r0:r0 + 6, :], start=True, stop=True)
            nc.vector.tensor_tensor(o_all[64 * i:64 * i + D, hp, :], bc[0:D, :],
                                    o_acc[0:D, h, :], mult)

    def emit_norm(hp):
        r = 32 * (hp // 3) + 2 * (hp % 3)
        nc.sync.dma_start(dens[r:r + 2, :],
                          o_acc[D:D + 1, 2 * hp:2 * hp + 2, :])

    def emit_scores(hp, kt):
        st = psS.tile([P, 2, 512], F32, tag="sc", name="sc")
        for i in range(2):
            h = 2 * hp + i
            g, m = h // 3, h % 3
            nc.tensor.matmul(st[:, i, :],
                             kT8[32 * m:32 * m + 32, :, g, kt * P:(kt + 1) * P],
                             qT8[32 * m:32 * m + 32, :, g, :],
                             start=True, stop=True, perf_mode=DR)
        ex = expool.tile([P, 2, 512], BF16, tag="ex")
        nc.scalar.activation(ex[:, :, :], st[:, :, :],
                             mybir.ActivationFunctionType.Exp, scale=1.0 / QS)
        return ex

    # --- wave-scheduled attention ---
    # Scores+exp are emitted in waves of 2 head pairs per kt-block; the
    # previous wave's PV matmuls, o-flushes, and remaining projection work
    # are pumped into the PE stream between score groups, so the Scalar
    # engine is fed continuously while PV never waits on exp.
    from collections import deque
    pending = deque()

    def pump(n=1):
        for _ in range(n):
            if pending:
                pending.popleft()()

    BLOCKS = [(0, 4), (4, 10), (10, 16)]
    LASTB = len(BLOCKS) - 1

    def make_pv(hp, kt, ex, o_ps, start, stop):
        def f():
            for i in range(2):
                nc.tensor.matmul(o_ps[i][:, :], v_sb[:, kt, 2 * hp + i, :], ex[:, i, :],
                                 start=start, stop=stop)
        return f

    def make_flush(b, hp, o_ps):
        def f():
            for i in range(2):
                h = 2 * hp + i
                if b == 0:
                    nc.vector.tensor_copy(o_acc[:, h, :], o_ps[i][:, :])
                else:
                    nc.vector.tensor_tensor(o_acc[:, h, :], o_ps[i][:, :], o_acc[:, h, :], add)
            if b == LASTB:
                emit_norm(hp)
        return f

    # startup: just enough projection for wave (hp0, hp1): q/k tiles t0..t3
    # (covers head groups g0/g1 = heads 0..5)
    done = set()

    def need_q(t):
        if ("q", t) not in done:
            done.add(("q", t))
            emit_qproj(t)

    def need_k(t):
        if ("k", t) not in done:
            done.add(("k", t))
            emit_kproj(t)

    def need_v(kt):
        if ("v", kt) not in done:
            done.add(("v", kt))
            emit_vproj(kt)

    for t in range(4):
        need_q(t)
    for t in range(4):
        need_k(t)

    BLOCKS = [(0, 4), (4, 10), (10, 16)]
    LASTB = len(BLOCKS) - 1

    def make_pv(hp, kt, ex, o_ps, start, stop):
        def f():
            for i in range(2):
                nc.tensor.matmul(o_ps[i][:, :], v_sb[:, kt, 2 * hp + i, :], ex[:, i, :],
                                 start=start, stop=stop)
        return f

    def make_flush(b, hp, o_ps):
        def f():
            for i in range(2):
                h = 2 * hp + i
                if b == 0:
                    nc.vector.tensor_copy(o_acc[:, h, :], o_ps[i][:, :])
                else:
                    nc.vector.tensor_tensor(o_acc[:, h, :], o_ps[i][:, :], o_acc[:, h, :], add)
            if b == LASTB:
                emit_norm(hp)
        return f

    # FIFO work queue pumped between score groups.  FIFO order guarantees a
    # block's v tiles (enqueued a block ahead) are emitted before that
    # block's PV matmuls; q/k tiles are deadline-pulled at wave starts via
    # the idempotent need_* helpers if their queued thunk hasn't popped yet.
    pending = deque()
    for kt in range(*BLOCKS[0]):
        pending.append(lambda kt=kt: need_v(kt))
    for t in range(4, PT):
        pending.append(lambda t=t: need_q(t))
        pending.append(lambda t=t: need_k(t))

    def pump():
        if pending:
            pending.popleft()()
        if len(pending) > 5 and pending:
            pending.popleft()()

    for b, (k0, k1) in enumerate(BLOCKS):
        if b < LASTB:
            for kt in range(*BLOCKS[b + 1]):
                pending.append(lambda kt=kt: need_v(kt))
        else:
            pending.append(lambda: nc.gpsimd.dma_start(
                wo_t[:], wo[:].rearrange("(ec p) m -> p ec m", p=P)))
        for w in range(3):
            for h in range(4 * w, 4 * w + 4):  # q/k tile deadline for this wave's heads
                g = h // 3
                need_q(2 * g)
                need_q(2 * g + 1)
                need_k(2 * g)
                need_k(2 * g + 1)
            for hp in (2 * w, 2 * w + 1):
                o_ps = [psO.tile([D + 1, 512], F32, tag="po", name=f"o{b}_{hp}_{i}")
                        for i in range(2)]
                exs = []
                for kt in range(k0, k1):
                    exs.append((kt, emit_scores(hp, kt)))
                    pump()
                for kt, ex in exs:
                    pending.append(make_pv(hp, kt, ex, o_ps,
                                           start=(kt == k0), stop=(kt == k1 - 1)))
                pending.append(make_flush(b, hp, o_ps))
                if b == LASTB and hp in (2, H // 2 - 1):
                    pending.append(lambda hp=hp: flush_norm(hp // 3))
    while pending:
        pending.popleft()()

    # --- output projection ---
    ST = QB // P
    for st4 in range(ST):
        op = psS.tile([P, 2, 512], F32, tag="sc", name="oproj")
        opf = op.rearrange("p a b -> p (a b)")
        for hp in range(H // 2):
            first = (hp == 0)
            last = (hp == H // 2 - 1)
            nc.tensor.matmul(opf[:, 0:512], o_all[:, hp, st4 * P:(st4 + 1) * P],
                             wo_t[:, hp, 0:512], start=first, stop=last)
            nc.tensor.matmul(opf[:, 512:768], o_all[:, hp, st4 * P:(st4 + 1) * P],
                             wo_t[:, hp, 512:768], start=first, stop=last)
        out_sb = outpool.tile([P, E], F32, tag="outsb")
        nc.vector.tensor_tensor(out_sb[:], opf[:, 0:768], bo_sb[:], add)
        nc.sync.dma_start(out[st4 * P:(st4 + 1) * P, :], out_sb[:])


_NC_CACHE = None


def _get_nc():
    global _NC_CACHE
    if _NC_CACHE is None:
        _NC_CACHE = build_nc()
    return _NC_CACHE


def _dsplit_perm():
    """col i = t*96 + m*32 + dm  <-  head (3*(t//2)+m), d (32*(t%2)+dm)."""
    perm = np.empty(E, dtype=np.int64)
    i = 0
    for t in range(PT):
        g, j = t // 2, t % 2
        for m in range(3):
            for dm in range(32):
                perm[i] = (3 * g + m) * D + 32 * j + dm
                i += 1
    return perm


def make_in_maps(query, key_, value, Wq, bq, Wk, bk, Wv, bv, Wo, bo):
    """Host-side sharding + layout prep. Returns list of 8 input dicts."""
    import ml_dtypes
    BF = ml_dtypes.bfloat16
    F8NP = mybir.dt.np(F8)

    query = np.asarray(query, dtype=np.float32)
    key_ = np.asarray(key_, dtype=np.float32)
    value = np.asarray(value, dtype=np.float32)
    scale = 1.0 / np.sqrt(np.float32(D))
    perm = _dsplit_perm()

    wq_eff = np.ascontiguousarray(np.transpose(np.asarray(Wq, np.float32), (1, 0, 2)).reshape(E, E)) * scale
    wk_eff = np.ascontiguousarray(np.transpose(np.asarray(Wk, np.float32), (1, 0, 2)).reshape(E, E))
    wq_f = np.ascontiguousarray(wq_eff[:, perm] * WS).astype(F8NP)
    wk_f = np.ascontiguousarray(wk_eff[:, perm] * WS).astype(F8NP)
    wv_f = np.ascontiguousarray(np.transpose(np.asarray(Wv, np.float32), (1, 0, 2)).reshape(E, E)).astype(BF)
    wo_f = np.ascontiguousarray(np.asarray(Wo, np.float32)).astype(BF)

    bq_eff = np.asarray(bq, np.float32).reshape(E) * scale * QS
    bk_eff = np.asarray(bk, np.float32).reshape(E)
    bq_f = np.zeros((P, PT), np.float32)
    bk_f = np.zeros((P, PT), np.float32)
    bq_f[0:PR, :] = bq_eff[perm].reshape(PT, PR).T
    bk_f[0:PR, :] = bk_eff[perm].reshape(PT, PR).T
    bv_f = np.asarray(bv, np.float32).reshape(E)
    bo_eff = np.tile((bv_f @ wo_f.astype(np.float32) + np.asarray(bo, np.float32)).reshape(1, E), (P, 1)).copy()

    sel_np = np.zeros((38, H * D), np.float32)
    for h in range(H):
        sel_np[32 * (h // 6) + h % 6, h * D:(h + 1) * D] = 1.0
    sel_np = sel_np.astype(BF)

    xk_t = [np.ascontiguousarray(key_[b].T).astype(F8NP) for b in range(B)]
    xv_t = [np.ascontiguousarray(value[b].T).astype(BF) for b in range(B)]

    in_maps = []
    for core in range(NCORES):
        b = core // (NCORES // B)
        qc = core % (NCORES // B)
        xq_f = np.ascontiguousarray(query[b, qc * QB:(qc + 1) * QB, :].T).astype(F8NP)
        in_maps.append({
            "xq": xq_f, "xk": xk_t[b], "xv": xv_t[b],
            "wq": wq_f, "wk": wk_f, "wv": wv_f, "wo": wo_f,
            "bq": bq_f, "bk": bk_f, "bo": bo_eff, "seld": sel_np,
        })
    return in_maps


def assemble(results):
    outp = np.empty((B, S, E), dtype=np.float32)
    for core in range(NCORES):
        b = core // (NCORES // B)
        qc = core % (NCORES // B)
        outp[b, qc * QB:(qc + 1) * QB, :] = results[core]["out"]
    return outp


def kernel(query, key_, value, Wq, bq, Wk, bk, Wv, bv, Wo, bo):
    nc = _get_nc()
    in_maps = make_in_maps(query, key_, value, Wq, bq, Wk, bk, Wv, bv, Wo, bo)
    res = run_bass_kernel_spmd(nc, in_maps, core_ids=list(range(NCORES)))
    return assemble(res.results)
